# revision 8
# baseline (speedup 1.0000x reference)
"""Causal self-attention Trainium2 kernel (B=2, T=4096, E=768, H=12, D=64).

Sharding: 8 cores = 2 batches x 4 head-groups (3 heads each). Each core:
  - computes q/k in transposed layout [d, t] (fp16) and v in natural layout
    [t, d] (fp16) for its 3 heads (fp32r projection matmuls, PE transposes
    for x^T),
  - causal attention in S^T layout ([key, query] tiles) so softmax
    normalization needs no P transposes: fp16 S/PV matmuls, exp on ACT
    writes fp16 directly, denominator via an extra ones-column appended to
    v (PV matmul row 64 = sum of exp),
  - normalizes via DVE reciprocal + PE broadcast + DVE mul,
    out-projects (fp32r) with its wo row-slice producing a partial
    y [4096, 768].
Host sums the 4 partials per batch and adds bo + bv @ wo (the v-bias
contributes a constant row after softmax normalization, so it never
touches the device).

The engine-time budget per core is PE ~255 us (matmuls at 1 col/cycle),
ACT ~225 us (exp of ~26M S entries), DVE ~140 us, Pool ~70 us. To keep PE
and ACT concurrently busy, projection/out-projection phases are
software-pipelined: phase1(c+1) and the out-projection of superblock c-1
are emitted in small chunks interleaved into attention(c)'s instruction
stream, so the PE fills its attention stalls (psum ping-pong paced by the
ACT exp) with projection matmuls instead of idling between superblocks.
"""

import os
import sys

sys.path.insert(0, "/opt/trn_rl_repo")

import numpy as np

try:  # persistent jit cache: skips neuronxcc compile on re-runs
    import jax

    jax.config.update("jax_compilation_cache_dir", "/tmp/jax_neff_cache")
    jax.config.update("jax_persistent_cache_min_compile_time_secs", 10)
    jax.config.update("jax_persistent_cache_min_entry_size_bytes", 0)
except Exception:
    pass

import concourse.bass as bass
import concourse.mybir as mybir
import concourse.tile as tile
from concourse import bacc
from concourse.bass_utils import run_bass_kernel_spmd

F32 = mybir.dt.float32
F32R = mybir.dt.float32r
F16 = mybir.dt.float16
U16 = mybir.dt.uint16

B, T, E, H = 2, 4096, 768, 12
D = E // H            # 64
HL = 3                # heads per core
CH = HL * D           # 192 channels per core
SB = 512              # query superblock
KB = 128              # key block
NEB = E // 128        # 6 embed tiles
SCALE = 1.0 / np.sqrt(D)
ONE_F16_BITS = 0x3C00  # 1.0 in fp16


def _mm(ap):
    return ap.bitcast(F32R)


def build_nc(t_len=T, repeat=1):
    assert t_len % SB == 0
    nsb = t_len // SB       # superblocks
    ntb = t_len // KB       # 128-blocks

    nc = bacc.Bacc("TRN2", target_bir_lowering=False, debug=False, num_devices=8)

    xb = nc.dram_tensor("xb", [t_len, E], F32, kind="ExternalInput")
    wqk = nc.dram_tensor("wqk", [E, 2 * CH], F32, kind="ExternalInput")
    wvp = nc.dram_tensor("wvp", [E, 256], F32, kind="ExternalInput")
    wo = nc.dram_tensor("wo", [CH, E], F32, kind="ExternalInput")
    bqk = nc.dram_tensor("bqk", [HL, 2, D], F32, kind="ExternalInput")
    cst = nc.dram_tensor("cst", [128, 192], F32, kind="ExternalInput")
    y = nc.dram_tensor("y", [t_len, E], F32, kind="ExternalOutput")

    xb, wqk, wvp, wo, bqk, cst, y = (
        t.ap() for t in (xb, wqk, wvp, wo, bqk, cst, y)
    )

    with tile.TileContext(nc) as tc:
        import contextlib

        ctx = contextlib.ExitStack()
        with ctx:
            ctx.enter_context(
                nc.allow_low_precision(reason="fp16/fp32r attention matmuls")
            )
            const = ctx.enter_context(tc.tile_pool(name="const", bufs=1))
            persist = ctx.enter_context(tc.tile_pool(name="persist", bufs=1))
            xpool = ctx.enter_context(tc.tile_pool(name="xpool", bufs=2))
            xtpool = ctx.enter_context(tc.tile_pool(name="xtpool", bufs=2))
            qspool = ctx.enter_context(tc.tile_pool(name="qspool", bufs=6))
            ospool = ctx.enter_context(tc.tile_pool(name="ospool", bufs=2))
            ptpool = ctx.enter_context(tc.tile_pool(name="ptpool", bufs=6))
            rpool = ctx.enter_context(tc.tile_pool(name="rpool", bufs=2))
            ypool = ctx.enter_context(tc.tile_pool(name="ypool", bufs=2))
            psA = ctx.enter_context(tc.tile_pool(name="psA", bufs=2, space="PSUM"))
            psS = ctx.enter_context(tc.tile_pool(name="psS", bufs=2, space="PSUM"))
            psO = ctx.enter_context(tc.tile_pool(name="psO", bufs=2, space="PSUM"))

            # ---- constants / weights in SBUF ----
            ident = const.tile([128, 128], F32)
            nc.sync.dma_start(out=_mm(ident), in_=_mm(cst[:, 0:128]))
            ones65 = const.tile([65, D], F32)
            nc.sync.dma_start(
                out=_mm(ones65[64:65, :]), in_=_mm(cst[64:65, 128 : 128 + D])
            )

            wqk_sb = const.tile([128, NEB, 2 * CH], F32)
            nc.sync.dma_start(
                out=_mm(wqk_sb), in_=_mm(wqk).rearrange("(n p) m -> p n m", p=128)
            )
            wv_sb = const.tile([128, NEB, 256], F32)
            nc.sync.dma_start(
                out=_mm(wv_sb), in_=_mm(wvp).rearrange("(n p) m -> p n m", p=128)
            )
            wo01_sb = const.tile([128, E], F32)
            nc.sync.dma_start(out=_mm(wo01_sb), in_=_mm(wo[0 : 2 * D, :]))
            wo2_sb = const.tile([D, E], F32)
            nc.sync.dma_start(out=_mm(wo2_sb), in_=_mm(wo[2 * D : CH, :]))
            bqk_sb = const.tile([D, HL, 2], F32)
            nc.sync.dma_start(out=bqk_sb, in_=bqk.rearrange("h q p -> p h q"))
            # k-bias copy living at partitions 64..127 (k rows of the packed
            # qk psum) so the staging add is partition-aligned
            bk64_sb = const.tile([128, HL], F32)
            nc.sync.dma_start(
                out=bk64_sb[D : 2 * D, :], in_=bqk[:, 1, :].rearrange("h p -> p h")
            )

            # persistent activations: kT [d, t] fp16; v natural fp16 with a
            # ones column at d=64 (softmax denominator via the PV matmul)
            kT = [persist.tile([D, t_len], F16, name=f"kT{h}") for h in range(HL)]
            v_sb = persist.tile([128, ntb, HL, D + 1], F16)
            # ones column is constant across iterations: write it once
            nc.vector.memset(
                v_sb[:, :, :, D : D + 1].bitcast(U16), ONE_F16_BITS
            )

            import contextlib as _cl

            loop_cm = tc.For_i(0, repeat, 1) if repeat > 1 else _cl.nullcontext()

            # ---------- phase builders (lists of emission thunks) ----------
            def phase1_chunks(c, qtiles):
                """x load, x^T, q/k/v (fp16) for superblock c. qtiles[h] is
                filled with the fp16 q tile when its chunk runs."""
                st = {}
                chunks = []

                def c_dma():
                    st["x"] = xpool.tile([128, 4, E], F32, tag="x", name="x_nat")
                    for tb in range(4):
                        t0 = c * SB + tb * KB
                        nc.sync.dma_start(
                            out=_mm(st["x"][:, tb, :]), in_=_mm(xb[t0 : t0 + KB, :])
                        )

                chunks.append(c_dma)

                def c_tr(eb):
                    if eb == 0:
                        st["xT"] = xtpool.tile(
                            [128, NEB, SB], F32, tag="xT", name="xT"
                        )
                    ps_t = psA.tile([128, SB], F32, tag="psA", name="ps_t")
                    for tb in range(4):
                        nc.tensor.transpose(
                            _mm(ps_t[:, tb * 128 : (tb + 1) * 128]),
                            _mm(st["x"][:, tb, eb * 128 : (eb + 1) * 128]),
                            _mm(ident),
                        )
                    nc.vector.tensor_copy(out=_mm(st["xT"][:, eb, :]), in_=ps_t)

                for eb in range(NEB):
                    chunks.append(lambda eb=eb: c_tr(eb))

                def c_qk(h):
                    ps_qk = psA.tile([128, SB], F32, tag="psA", name="ps_qk")
                    for eb in range(NEB):
                        nc.tensor.matmul(
                            ps_qk,
                            lhsT=_mm(wqk_sb[:, eb, h * 128 : (h + 1) * 128]),
                            rhs=_mm(st["xT"][:, eb, :]),
                            start=(eb == 0),
                            stop=(eb == NEB - 1),
                        )
                    q_h = qspool.tile([D, SB], F16, tag="qS", name="q_h")
                    nc.vector.tensor_scalar_add(
                        out=q_h, in0=ps_qk[0:D, :], scalar1=bqk_sb[:, h, 0:1]
                    )
                    qtiles[h] = q_h
                    # k rows live at psum partitions 64..127. Lane engines
                    # cannot shift partitions, so stage at the same partitions
                    # (adding bias) and let an SBUF->SBUF DMA move them to
                    # partition base 0 in kT.
                    kst = qspool.tile([128, SB], F16, tag="kst", name="kst", bufs=2)
                    nc.vector.tensor_scalar_add(
                        out=kst[D : 2 * D, :],
                        in0=ps_qk[D : 2 * D, :],
                        scalar1=bk64_sb[D : 2 * D, h : h + 1],
                    )
                    nc.sync.dma_start(
                        out=kT[h][:, c * SB : (c + 1) * SB],
                        in_=kst[D : 2 * D, :],
                    )

                for h in range(HL):
                    chunks.append(lambda h=h: c_qk(h))

                def c_v(tb):
                    j = c * 4 + tb
                    ps_v = psA.tile([128, 256], F32, tag="psA", name="ps_v")
                    for eb in range(NEB):
                        nc.tensor.matmul(
                            ps_v,
                            lhsT=_mm(st["xT"][:, eb, tb * 128 : (tb + 1) * 128]),
                            rhs=_mm(wv_sb[:, eb, :]),
                            start=(eb == 0),
                            stop=(eb == NEB - 1),
                        )
                    nc.vector.tensor_copy(
                        out=v_sb[:, j, :, 0:D],
                        in_=ps_v[:, 0:CH].rearrange("p (h d) -> p h d", h=HL),
                    )

                for tb in range(4):
                    chunks.append(lambda tb=tb: c_v(tb))
                return chunks

            def phase3_chunks(cp, oS):
                """out-projection of superblock cp -> partial y rows."""
                oS01p, oS2p = oS
                st = {}
                chunks = []

                def c_half(tb, half):
                    if half == 0:
                        st[tb] = ypool.tile([128, E], F32, tag="y_sb", name="y_sb")
                    y_sb = st[tb]
                    ps_y = psA.tile([128, 384], F32, tag="psA", name="ps_y")
                    nc.tensor.matmul(
                        ps_y,
                        lhsT=_mm(oS01p[:, tb * KB : (tb + 1) * KB]),
                        rhs=_mm(wo01_sb[:, half * 384 : (half + 1) * 384]),
                        start=True,
                        stop=False,
                    )
                    nc.tensor.matmul(
                        ps_y,
                        lhsT=_mm(oS2p[:, tb * KB : (tb + 1) * KB]),
                        rhs=_mm(wo2_sb[:, half * 384 : (half + 1) * 384]),
                        start=False,
                        stop=True,
                    )
                    nc.vector.tensor_copy(
                        out=y_sb[:, half * 384 : (half + 1) * 384], in_=ps_y
                    )
                    if half == 1:
                        tg = cp * 4 + tb
                        nc.sync.dma_start(
                            out=y[tg * KB : (tg + 1) * KB, :], in_=y_sb
                        )

                for tb in range(4):
                    for half in range(2):
                        chunks.append(lambda tb=tb, half=half: c_half(tb, half))
                return chunks

            with loop_cm:
                # prologue: superblock 0's projections emitted directly
                qS_cur = [None] * HL
                for ch in phase1_chunks(0, qS_cur):
                    ch()
                oS_prev = None

                for c in range(nsb):
                    nj = 4 * c + 4
                    npr = nj // 2

                    # background work to interleave into this attention block
                    qS_next = [None] * HL
                    bg = []
                    if c > 0:
                        bg += phase3_chunks(c - 1, oS_prev)
                    if c + 1 < nsb:
                        bg += phase1_chunks(c + 1, qS_next)
                    bg_total = len(bg)
                    bg_done = 0
                    steps_total = 3 * npr
                    step_i = 0

                    def bg_step():
                        nonlocal bg_done, step_i
                        step_i += 1
                        want = bg_total * step_i // steps_total
                        while bg_done < min(want, bg_total):
                            bg[bg_done]()
                            bg_done += 1

                    def bg_flush():
                        nonlocal bg_done
                        while bg_done < bg_total:
                            bg[bg_done]()
                            bg_done += 1

                    oS01 = ospool.tile([128, SB], F32, tag="oS01", name="oS01")
                    oS2 = ospool.tile([D, SB], F32, tag="oS2", name="oS2")
                    oS_now = (oS01, oS2)

                    def norm_chain(h, ps_o):
                        # PV(h) -> DVE recip -> Pool partition-broadcast ->
                        # DVE mul
                        recip = rpool.tile([65, SB], F32, tag="recip", name="recip")
                        nc.vector.reciprocal(
                            _mm(recip[64:65, :]), ps_o[D : D + 1, :]
                        )
                        # psA slots are idle during attention: use one for
                        # the broadcast so the psS rotation is untouched
                        ps_b = psA.tile([128, SB], F32, tag="psA", name="ps_b")
                        nc.tensor.matmul(
                            ps_b[0:D, :],
                            lhsT=_mm(ones65[64:65, :]),
                            rhs=_mm(recip[64:65, :]),
                            start=True,
                            stop=True,
                        )
                        # walrus: a DVE op may read only ONE non-scalar PSUM
                        # input, so stage the broadcast row in SBUF
                        rb = rpool.tile([D, SB], F32, tag="rbcast", name="rb")
                        nc.vector.tensor_copy(out=rb, in_=ps_b[0:D, :])
                        if h == 0:
                            o_dst = oS01[0:D, :]
                        elif h == 2:
                            o_dst = oS2[:, :]
                        else:
                            o_dst = ospool.tile(
                                [D, SB], F32, tag="o1tmp", name="o1tmp"
                            )
                        nc.vector.tensor_mul(_mm(o_dst), ps_o[0:D, :], rb)
                        if h == 1:
                            # stack h1 under h0 (partitions 64:128) via DMA,
                            # the only engine that can shift partitions
                            nc.sync.dma_start(
                                out=_mm(oS01[D : 2 * D, :]), in_=_mm(o_dst)
                            )

                    def q0_of(j):
                        # causal slice: key block j only sees queries
                        # >= j*KB - c*SB; pairs share the earlier block's q0
                        if j < 4 * c:
                            return 0
                        return min((j - 4 * c) * KB, SB - 256)

                    class HeadStream:
                        # one head's attention as a pair-granular generator:
                        # 2 fp16 S matmuls -> one exp (fp16 out) -> causal
                        # mask -> per-block fp16 PV matmuls (lagged so the
                        # PE never waits on the ACT->Pool turnaround)
                        def __init__(self, h):
                            self.h = h
                            self.ps_o = psO.tile(
                                [128, SB], F32, tag="psO", name="ps_o"
                            )
                            self.pend = []

                        def pv_step(self, j, pt_ap, q0):
                            nc.tensor.matmul(
                                self.ps_o[0 : D + 1, q0:SB],
                                lhsT=v_sb[:, j, self.h, :],
                                rhs=pt_ap[:, q0:SB],
                                start=(j == 0),
                                stop=(j == nj - 1),
                            )

                        def pair(self, jp, depth):
                            h = self.h
                            j0, j1 = 2 * jp, 2 * jp + 1
                            q0 = q0_of(j0)
                            ps_s2 = psS.tile(
                                [128, 2, SB], F32, tag="psS", name="ps_s2"
                            )
                            pt2 = ptpool.tile(
                                [128, 2, SB], F16, tag="pt", name="pt2"
                            )
                            for half, j in ((0, j0), (1, j1)):
                                nc.tensor.matmul(
                                    ps_s2[:, half, q0:SB],
                                    lhsT=kT[h][:, j * KB : (j + 1) * KB],
                                    rhs=qS_cur[h][:, q0:SB],
                                    start=True,
                                    stop=True,
                                )
                            nc.scalar.activation(
                                out=pt2[:, :, q0:],
                                in_=ps_s2[:, :, q0:],
                                func=mybir.ActivationFunctionType.Exp,
                                scale=float(SCALE),
                            )
                            for half, j in ((0, j0), (1, j1)):
                                if j >= 4 * c:
                                    nc.gpsimd.affine_select(
                                        out=pt2[:, half, q0:],
                                        in_=pt2[:, half, q0:],
                                        compare_op=mybir.AluOpType.is_ge,
                                        fill=0.0,
                                        base=c * SB - j * KB + q0,
                                        pattern=[[1, SB - q0]],
                                        channel_multiplier=-1,
                                    )
                            self.pend.append((j0, pt2[:, 0, :], q0))
                            self.pend.append((j1, pt2[:, 1, :], q0))
                            while len(self.pend) > depth:
                                self.pv_step(*self.pend.pop(0))

                        def drain(self):
                            for t in self.pend:
                                self.pv_step(*t)
                            self.pend = []

                    # heads 0 and 1 interleave pair-by-pair; head 2 runs
                    # alone with a deeper private pipeline. background
                    # projection chunks are spread across the whole block.
                    s0, s1 = HeadStream(0), HeadStream(1)
                    for jp in range(npr):
                        s0.pair(jp, 2)
                        bg_step()
                        s1.pair(jp, 2)
                        bg_step()
                    s0.drain()
                    s1.drain()
                    norm_chain(0, s0.ps_o)
                    s2 = HeadStream(2)
                    for jp in range(npr):
                        s2.pair(jp, 6)
                        bg_step()
                    norm_chain(1, s1.ps_o)
                    bg_flush()
                    s2.drain()
                    norm_chain(2, s2.ps_o)
                    oS_prev = oS_now
                    qS_cur = qS_next if c + 1 < nsb else qS_cur
                for ch in phase3_chunks(nsb - 1, oS_prev):
                    ch()
    nc.compile()
    return nc


def make_in_maps(x, wq, bq, wk, bk, wv, bv, wo, bo, t_len=T):
    x = np.asarray(x, np.float32)
    in_maps = []
    for c in range(8):
        b, g = divmod(c, 4)
        hs = slice(g * CH, (g + 1) * CH)
        wqk_c = np.empty((E, 2 * CH), np.float32)
        bqk_c = np.empty((HL, 2, D), np.float32)
        for hl in range(HL):
            h = g * HL + hl
            wqk_c[:, hl * 128 : hl * 128 + D] = wq[:, h * D : (h + 1) * D]
            wqk_c[:, hl * 128 + D : (hl + 1) * 128] = wk[:, h * D : (h + 1) * D]
            bqk_c[hl, 0] = bq[h * D : (h + 1) * D]
            bqk_c[hl, 1] = bk[h * D : (h + 1) * D]
        wv_c = np.zeros((E, 256), np.float32)
        wv_c[:, :CH] = np.asarray(wv, np.float32)[:, hs]
        cst = np.concatenate(
            [np.eye(128, dtype=np.float32), np.ones((128, 64), np.float32)], axis=1
        )
        in_maps.append(
            {
                "xb": np.ascontiguousarray(x[b, :t_len]),
                "wqk": wqk_c,
                "wvp": wv_c,
                "wo": np.ascontiguousarray(np.asarray(wo, np.float32)[hs]),
                "bqk": bqk_c,
                "cst": cst,
            }
        )
    return in_maps


def host_combine(y_per_core, bo, bv, wo):
    """Sum the 4 per-core partial y's per batch; add bo and the v-bias
    contribution bv @ wo (constant after softmax normalization)."""
    bias_row = (
        np.asarray(bv, np.float32) @ np.asarray(wo, np.float32)
        + np.asarray(bo, np.float32)
    )
    out = np.empty((B, T, E), np.float32)
    for b in range(B):
        acc = y_per_core[b * 4].astype(np.float32).copy()
        for g in range(1, 4):
            acc += y_per_core[b * 4 + g]
        out[b] = acc + bias_row
    return out


_NC_CACHE = {}


def get_nc(t_len=T):
    if t_len not in _NC_CACHE:
        _NC_CACHE[t_len] = build_nc(t_len)
    return _NC_CACHE[t_len]


def _build_sharded_nodonate(nc, n_cores=8):
    """Mirror bass2jax.run_bass_via_pjrt's multi-core path, minus donation,
    returning (jitted_fn, in_names, out_names, out_avals). Without donation a
    call can be repeated on device-resident arrays for timing. Safe here: the
    kernel writes every element of y."""
    import jax
    from jax.sharding import Mesh, PartitionSpec
    from jax.experimental.shard_map import shard_map

    from concourse import bass2jax
    from concourse.bass2jax import _bass_exec_p

    bass2jax.install_neuronx_cc_hook()
    part_name = nc.partition_id_tensor.name if nc.partition_id_tensor else None

    in_names, out_names, out_avals = [], [], []
    for alloc in nc.m.functions[0].allocations:
        if not isinstance(alloc, mybir.MemoryLocationSet):
            continue
        name = alloc.memorylocations[0].name
        if alloc.kind == "ExternalInput":
            if name != part_name:
                in_names.append(name)
        elif alloc.kind == "ExternalOutput":
            shape = tuple(alloc.tensor_shape)
            dtype = mybir.dt.np(alloc.dtype)
            out_names.append(name)
            out_avals.append(jax.core.ShapedArray(shape, dtype))
    n_params = len(in_names)
    all_names = in_names + out_names
    if part_name is not None:
        all_names = all_names + [part_name]

    def _body(*args):
        operands = list(args)
        if part_name is not None:
            operands.append(bass2jax.partition_id_tensor())
        outs = _bass_exec_p.bind(
            *operands,
            out_avals=tuple(out_avals),
            in_names=tuple(all_names),
            out_names=tuple(out_names),
            lowering_input_output_aliases=(),
            sim_require_finite=True,
            sim_require_nnan=True,
            nc=nc,
        )
        return tuple(outs)

    devices = jax.devices()[:n_cores]
    mesh = Mesh(np.asarray(devices), ("core",))
    n_out = len(out_names)
    sharded = jax.jit(
        shard_map(
            _body,
            mesh=mesh,
            in_specs=(PartitionSpec("core"),) * (n_params + n_out),
            out_specs=(PartitionSpec("core"),) * n_out,
            check_rep=False,
        ),
        keep_unused=True,
    )
    return sharded, in_names, out_names, out_avals


def run_timed(nc, in_maps, iters=20):
    """Execute on HW repeatedly with device-resident args; returns
    (per-core results, sorted per-call walls in seconds)."""
    import time

    import jax

    n_cores = len(in_maps)
    sharded, in_names, out_names, out_avals = _build_sharded_nodonate(nc, n_cores)
    concat_in = [
        np.concatenate([np.asarray(m[name]) for m in in_maps], axis=0)
        for name in in_names
    ]
    concat_zero = [
        np.zeros((n_cores * a.shape[0], *a.shape[1:]), a.dtype) for a in out_avals
    ]
    args = [jax.device_put(a) for a in concat_in + concat_zero]
    out = sharded(*args)  # compile + first run
    jax.block_until_ready(out)
    walls = []
    for _ in range(iters):
        t0 = time.perf_counter()
        out2 = sharded(*args)
        jax.block_until_ready(out2)
        walls.append(time.perf_counter() - t0)
    results = [
        {
            name: np.asarray(out[i]).reshape(n_cores, *out_avals[i].shape)[c]
            for i, name in enumerate(out_names)
        }
        for c in range(n_cores)
    ]
    return results, sorted(walls)


def baseline_rtt(iters=20):
    """Axon dispatch floor: same path with a trivial 8-core kernel."""
    nc = bacc.Bacc("TRN2", target_bir_lowering=False, debug=False, num_devices=8)
    a = nc.dram_tensor("a", [128, 128], F32, kind="ExternalInput")
    b = nc.dram_tensor("b", [128, 128], F32, kind="ExternalOutput")
    a, b = a.ap(), b.ap()
    with tile.TileContext(nc) as tc:
        with tc.tile_pool(name="p", bufs=1) as p:
            t = p.tile([128, 128], F32)
            nc.sync.dma_start(out=t, in_=a)
            nc.scalar.mul(out=t, in_=t, mul=2.0)
            nc.sync.dma_start(out=b, in_=t)
    nc.compile()
    in_maps = [{"a": np.zeros((128, 128), np.float32)} for _ in range(8)]
    _, walls = run_timed(nc, in_maps, iters=iters)
    return walls


def kernel(x, wq, bq, wk, bk, wv, bv, wo, bo, _trace=False, _trace_kwargs=None):
    nc = get_nc()
    in_maps = make_in_maps(x, wq, bq, wk, bk, wv, bv, wo, bo)
    res = run_bass_kernel_spmd(
        nc, in_maps, list(range(8)), trace=_trace, **(_trace_kwargs or {})
    )
    out = host_combine([res.results[c]["y"] for c in range(8)], bo, bv, wo)
    if _trace:
        return out, res
    return out


# revision 9
# speedup vs baseline: 1.1883x; 1.1883x over previous
"""Causal self-attention Trainium2 kernel (B=2, T=4096, E=768, H=12, D=64).

Sharding: 8 cores = 2 batches x 4 head-groups (3 heads each). Each core:
  - computes q/k in transposed layout [d, t] (fp16) and v in natural layout
    [t, d] (fp16) for its 3 heads (fp32r projection matmuls, PE transposes
    for x^T),
  - causal attention in S^T layout ([key, query] tiles) so softmax
    normalization needs no P transposes: fp16 S/PV matmuls, exp on ACT
    writes fp16 directly, denominator via an extra ones-column appended to
    v (PV matmul row 64 = sum of exp),
  - normalizes via DVE reciprocal + PE broadcast + DVE mul,
    out-projects (fp32r) with its wo row-slice producing a partial
    y [4096, 768].
Host sums the 4 partials per batch and adds bo + bv @ wo (the v-bias
contributes a constant row after softmax normalization, so it never
touches the device).

The engine-time budget per core is PE ~255 us (matmuls at 1 col/cycle),
ACT ~225 us (exp of ~26M S entries), DVE ~140 us, Pool ~70 us. To keep PE
and ACT concurrently busy, projection/out-projection phases are
software-pipelined: phase1(c+1) and the out-projection of superblock c-1
are emitted in small chunks interleaved into attention(c)'s instruction
stream, so the PE fills its attention stalls (psum ping-pong paced by the
ACT exp) with projection matmuls instead of idling between superblocks.
"""

import os
import sys

sys.path.insert(0, "/opt/trn_rl_repo")

import numpy as np

try:  # persistent jit cache: skips neuronxcc compile on re-runs
    import jax

    jax.config.update("jax_compilation_cache_dir", "/tmp/jax_neff_cache")
    jax.config.update("jax_persistent_cache_min_compile_time_secs", 10)
    jax.config.update("jax_persistent_cache_min_entry_size_bytes", 0)
except Exception:
    pass

import concourse.bass as bass
import concourse.mybir as mybir
import concourse.tile as tile
from concourse import bacc
from concourse.bass_utils import run_bass_kernel_spmd

F32 = mybir.dt.float32
F32R = mybir.dt.float32r
F16 = mybir.dt.float16
U16 = mybir.dt.uint16

B, T, E, H = 2, 4096, 768, 12
D = E // H            # 64
HL = 3                # heads per core
CH = HL * D           # 192 channels per core
SB = 512              # query superblock
KB = 128              # key block
NEB = E // 128        # 6 embed tiles
SCALE = 1.0 / np.sqrt(D)
ONE_F16_BITS = 0x3C00  # 1.0 in fp16


def _mm(ap):
    return ap.bitcast(F32R)


def build_nc(t_len=T, repeat=1):
    assert t_len % SB == 0
    nsb = t_len // SB       # superblocks
    ntb = t_len // KB       # 128-blocks

    nc = bacc.Bacc("TRN2", target_bir_lowering=False, debug=False, num_devices=8)

    xb = nc.dram_tensor("xb", [t_len, E], F32, kind="ExternalInput")
    wqk = nc.dram_tensor("wqk", [E, 2 * CH], F32, kind="ExternalInput")
    wvp = nc.dram_tensor("wvp", [E, 256], F32, kind="ExternalInput")
    wo = nc.dram_tensor("wo", [CH, E], F32, kind="ExternalInput")
    bqk = nc.dram_tensor("bqk", [HL, 2, D], F32, kind="ExternalInput")
    cst = nc.dram_tensor("cst", [128, 192], F32, kind="ExternalInput")
    msk = nc.dram_tensor("msk", [128, 2, SB], F16, kind="ExternalInput")
    y = nc.dram_tensor("y", [t_len, E], F32, kind="ExternalOutput")

    xb, wqk, wvp, wo, bqk, cst, msk, y = (
        t.ap() for t in (xb, wqk, wvp, wo, bqk, cst, msk, y)
    )

    with tile.TileContext(nc) as tc:
        import contextlib

        ctx = contextlib.ExitStack()
        with ctx:
            ctx.enter_context(
                nc.allow_low_precision(reason="fp16/fp32r attention matmuls")
            )
            const = ctx.enter_context(tc.tile_pool(name="const", bufs=1))
            persist = ctx.enter_context(tc.tile_pool(name="persist", bufs=1))
            xpool = ctx.enter_context(tc.tile_pool(name="xpool", bufs=2))
            xtpool = ctx.enter_context(tc.tile_pool(name="xtpool", bufs=2))
            qspool = ctx.enter_context(tc.tile_pool(name="qspool", bufs=6))
            ospool = ctx.enter_context(tc.tile_pool(name="ospool", bufs=2))
            ptpool = ctx.enter_context(tc.tile_pool(name="ptpool", bufs=6))
            rpool = ctx.enter_context(tc.tile_pool(name="rpool", bufs=2))
            ypool = ctx.enter_context(tc.tile_pool(name="ypool", bufs=2))
            psA = ctx.enter_context(tc.tile_pool(name="psA", bufs=2, space="PSUM"))
            psS = ctx.enter_context(tc.tile_pool(name="psS", bufs=2, space="PSUM"))
            psO = ctx.enter_context(tc.tile_pool(name="psO", bufs=2, space="PSUM"))

            # ---- constants / weights in SBUF ----
            ident = const.tile([128, 128], F32)
            nc.sync.dma_start(out=_mm(ident), in_=_mm(cst[:, 0:128]))
            ones65 = const.tile([65, D], F32)
            nc.sync.dma_start(
                out=_mm(ones65[64:65, :]), in_=_mm(cst[64:65, 128 : 128 + D])
            )

            wqk_sb = const.tile([128, NEB, 2 * CH], F32)
            nc.sync.dma_start(
                out=_mm(wqk_sb), in_=_mm(wqk).rearrange("(n p) m -> p n m", p=128)
            )
            wv_sb = const.tile([128, NEB, 256], F32)
            nc.sync.dma_start(
                out=_mm(wv_sb), in_=_mm(wvp).rearrange("(n p) m -> p n m", p=128)
            )
            wo01_sb = const.tile([128, E], F32)
            nc.sync.dma_start(out=_mm(wo01_sb), in_=_mm(wo[0 : 2 * D, :]))
            wo2_sb = const.tile([D, E], F32)
            nc.sync.dma_start(out=_mm(wo2_sb), in_=_mm(wo[2 * D : CH, :]))
            bqk_sb = const.tile([D, HL, 2], F32)
            nc.sync.dma_start(out=bqk_sb, in_=bqk.rearrange("h q p -> p h q"))
            # k-bias copy living at partitions 64..127 (k rows of the packed
            # qk psum) so the staging add is partition-aligned
            bk64_sb = const.tile([128, HL], F32)
            nc.sync.dma_start(
                out=bk64_sb[D : 2 * D, :], in_=bqk[:, 1, :].rearrange("h p -> p h")
            )

            # causal masks for the two diagonal-block offsets:
            # msk_sb[:, 0, c] = (c >= k), msk_sb[:, 1, c] = (c >= k + 128)
            msk_sb = const.tile([128, 2, SB], F16)
            nc.sync.dma_start(out=msk_sb, in_=msk)

            # persistent activations: kT [d, t] fp16; v natural fp16 with a
            # ones column at d=64 (softmax denominator via the PV matmul)
            kT = [persist.tile([D, t_len], F16, name=f"kT{h}") for h in range(HL)]
            v_sb = persist.tile([128, ntb, HL, D + 1], F16)
            # ones column is constant across iterations: write it once
            nc.vector.memset(
                v_sb[:, :, :, D : D + 1].bitcast(U16), ONE_F16_BITS
            )

            import contextlib as _cl

            loop_cm = tc.For_i(0, repeat, 1) if repeat > 1 else _cl.nullcontext()

            # ---------- phase builders (lists of emission thunks) ----------
            def phase1_chunks(c, qtiles):
                """x load, x^T, q/k/v (fp16) for superblock c. qtiles[h] is
                filled with the fp16 q tile when its chunk runs."""
                st = {}
                chunks = []

                def c_dma():
                    st["x"] = xpool.tile([128, 4, E], F32, tag="x", name="x_nat")
                    for tb in range(4):
                        t0 = c * SB + tb * KB
                        nc.sync.dma_start(
                            out=_mm(st["x"][:, tb, :]), in_=_mm(xb[t0 : t0 + KB, :])
                        )

                chunks.append(c_dma)

                def c_tr(eb):
                    if eb == 0:
                        st["xT"] = xtpool.tile(
                            [128, NEB, SB], F32, tag="xT", name="xT"
                        )
                    ps_t = psA.tile([128, SB], F32, tag="psA", name="ps_t")
                    for tb in range(4):
                        nc.tensor.transpose(
                            _mm(ps_t[:, tb * 128 : (tb + 1) * 128]),
                            _mm(st["x"][:, tb, eb * 128 : (eb + 1) * 128]),
                            _mm(ident),
                        )
                    nc.vector.tensor_copy(out=_mm(st["xT"][:, eb, :]), in_=ps_t)

                for eb in range(NEB):
                    chunks.append(lambda eb=eb: c_tr(eb))

                def c_qk(h):
                    ps_qk = psA.tile([128, SB], F32, tag="psA", name="ps_qk")
                    for eb in range(NEB):
                        nc.tensor.matmul(
                            ps_qk,
                            lhsT=_mm(wqk_sb[:, eb, h * 128 : (h + 1) * 128]),
                            rhs=_mm(st["xT"][:, eb, :]),
                            start=(eb == 0),
                            stop=(eb == NEB - 1),
                        )
                    q_h = qspool.tile([D, SB], F16, tag="qS", name="q_h")
                    nc.vector.tensor_scalar_add(
                        out=q_h, in0=ps_qk[0:D, :], scalar1=bqk_sb[:, h, 0:1]
                    )
                    qtiles[h] = q_h
                    # k rows live at psum partitions 64..127. Lane engines
                    # cannot shift partitions, so stage at the same partitions
                    # (adding bias) and let an SBUF->SBUF DMA move them to
                    # partition base 0 in kT.
                    kst = qspool.tile([128, SB], F16, tag="kst", name="kst", bufs=2)
                    nc.vector.tensor_scalar_add(
                        out=kst[D : 2 * D, :],
                        in0=ps_qk[D : 2 * D, :],
                        scalar1=bk64_sb[D : 2 * D, h : h + 1],
                    )
                    nc.sync.dma_start(
                        out=kT[h][:, c * SB : (c + 1) * SB],
                        in_=kst[D : 2 * D, :],
                    )

                for h in range(HL):
                    chunks.append(lambda h=h: c_qk(h))

                def c_v(tb):
                    j = c * 4 + tb
                    ps_v = psA.tile([128, 256], F32, tag="psA", name="ps_v")
                    for eb in range(NEB):
                        nc.tensor.matmul(
                            ps_v,
                            lhsT=_mm(st["xT"][:, eb, tb * 128 : (tb + 1) * 128]),
                            rhs=_mm(wv_sb[:, eb, :]),
                            start=(eb == 0),
                            stop=(eb == NEB - 1),
                        )
                    nc.vector.tensor_copy(
                        out=v_sb[:, j, :, 0:D],
                        in_=ps_v[:, 0:CH].rearrange("p (h d) -> p h d", h=HL),
                    )

                for tb in range(4):
                    chunks.append(lambda tb=tb: c_v(tb))
                return chunks

            def phase3_chunks(cp, oS):
                """out-projection of superblock cp -> partial y rows."""
                oS01p, oS2p = oS
                st = {}
                chunks = []

                def c_half(tb, half):
                    if half == 0:
                        st[tb] = ypool.tile([128, E], F32, tag="y_sb", name="y_sb")
                    y_sb = st[tb]
                    ps_y = psA.tile([128, 384], F32, tag="psA", name="ps_y")
                    nc.tensor.matmul(
                        ps_y,
                        lhsT=_mm(oS01p[:, tb * KB : (tb + 1) * KB]),
                        rhs=_mm(wo01_sb[:, half * 384 : (half + 1) * 384]),
                        start=True,
                        stop=False,
                    )
                    nc.tensor.matmul(
                        ps_y,
                        lhsT=_mm(oS2p[:, tb * KB : (tb + 1) * KB]),
                        rhs=_mm(wo2_sb[:, half * 384 : (half + 1) * 384]),
                        start=False,
                        stop=True,
                    )
                    nc.vector.tensor_copy(
                        out=y_sb[:, half * 384 : (half + 1) * 384], in_=ps_y
                    )
                    if half == 1:
                        tg = cp * 4 + tb
                        nc.sync.dma_start(
                            out=y[tg * KB : (tg + 1) * KB, :], in_=y_sb
                        )

                for tb in range(4):
                    for half in range(2):
                        chunks.append(lambda tb=tb, half=half: c_half(tb, half))
                return chunks

            with loop_cm:
                # prologue: superblock 0's projections emitted directly
                qS_cur = [None] * HL
                for ch in phase1_chunks(0, qS_cur):
                    ch()
                oS_prev = None

                for c in range(nsb):
                    nj = 4 * c + 4
                    npr = nj // 2

                    # background work to interleave into this attention block
                    qS_next = [None] * HL
                    bg = []
                    if c > 0:
                        bg += phase3_chunks(c - 1, oS_prev)
                    if c + 1 < nsb:
                        bg += phase1_chunks(c + 1, qS_next)
                    bg_total = len(bg)
                    bg_done = 0
                    steps_total = 3 * npr
                    step_i = 0

                    def bg_step():
                        nonlocal bg_done, step_i
                        step_i += 1
                        want = bg_total * step_i // steps_total
                        while bg_done < min(want, bg_total):
                            bg[bg_done]()
                            bg_done += 1

                    def bg_flush():
                        nonlocal bg_done
                        while bg_done < bg_total:
                            bg[bg_done]()
                            bg_done += 1

                    oS01 = ospool.tile([128, SB], F32, tag="oS01", name="oS01")
                    oS2 = ospool.tile([D, SB], F32, tag="oS2", name="oS2")
                    oS_now = (oS01, oS2)

                    def norm_chain(h, ps_o):
                        # PV(h) -> DVE recip -> Pool partition-broadcast ->
                        # DVE mul
                        recip = rpool.tile([65, SB], F32, tag="recip", name="recip")
                        nc.vector.reciprocal(
                            _mm(recip[64:65, :]), ps_o[D : D + 1, :]
                        )
                        # psA slots are idle during attention: use one for
                        # the broadcast so the psS rotation is untouched
                        ps_b = psA.tile([128, SB], F32, tag="psA", name="ps_b")
                        nc.tensor.matmul(
                            ps_b[0:D, :],
                            lhsT=_mm(ones65[64:65, :]),
                            rhs=_mm(recip[64:65, :]),
                            start=True,
                            stop=True,
                        )
                        # walrus: a DVE op may read only ONE non-scalar PSUM
                        # input, so stage the broadcast row in SBUF
                        rb = rpool.tile([D, SB], F32, tag="rbcast", name="rb")
                        nc.vector.tensor_copy(out=rb, in_=ps_b[0:D, :])
                        if h == 0:
                            o_dst = oS01[0:D, :]
                        elif h == 2:
                            o_dst = oS2[:, :]
                        else:
                            o_dst = ospool.tile(
                                [D, SB], F32, tag="o1tmp", name="o1tmp"
                            )
                        nc.vector.tensor_mul(_mm(o_dst), ps_o[0:D, :], rb)
                        if h == 1:
                            # stack h1 under h0 (partitions 64:128) via DMA,
                            # the only engine that can shift partitions
                            nc.sync.dma_start(
                                out=_mm(oS01[D : 2 * D, :]), in_=_mm(o_dst)
                            )

                    def q0_of(j):
                        # causal slice: key block j only sees queries
                        # >= j*KB - c*SB; pairs share the earlier block's q0
                        if j < 4 * c:
                            return 0
                        return min((j - 4 * c) * KB, SB - 256)

                    class HeadStream:
                        # one head's attention as a pair-granular generator:
                        # 2 fp16 S matmuls -> one exp (fp16 out) -> causal
                        # mask -> per-block fp16 PV matmuls (lagged so the
                        # PE never waits on the ACT->Pool turnaround)
                        def __init__(self, h):
                            self.h = h
                            self.ps_o = psO.tile(
                                [128, SB], F32, tag="psO", name="ps_o"
                            )
                            self.pend = []

                        def pv_step(self, j, pt_ap, q0):
                            nc.tensor.matmul(
                                self.ps_o[0 : D + 1, q0:SB],
                                lhsT=v_sb[:, j, self.h, :],
                                rhs=pt_ap[:, q0:SB],
                                start=(j == 0),
                                stop=(j == nj - 1),
                            )

                        def pair(self, jp, depth):
                            h = self.h
                            j0, j1 = 2 * jp, 2 * jp + 1
                            q0 = q0_of(j0)
                            ps_s2 = psS.tile(
                                [128, 2, SB], F32, tag="psS", name="ps_s2"
                            )
                            pt2 = ptpool.tile(
                                [128, 2, SB], F16, tag="pt", name="pt2"
                            )
                            for half, j in ((0, j0), (1, j1)):
                                nc.tensor.matmul(
                                    ps_s2[:, half, q0:SB],
                                    lhsT=kT[h][:, j * KB : (j + 1) * KB],
                                    rhs=qS_cur[h][:, q0:SB],
                                    start=True,
                                    stop=True,
                                )
                            nc.scalar.activation(
                                out=pt2[:, :, q0:],
                                in_=ps_s2[:, :, q0:],
                                func=mybir.ActivationFunctionType.Exp,
                                scale=float(SCALE),
                            )
                            for half, j in ((0, j0), (1, j1)):
                                if j >= 4 * c:
                                    # causal mask: DVE multiply by the
                                    # precomputed 0/1 fp16 mask for this
                                    # block offset (j*KB - c*SB - q0 is 0
                                    # or 128 under pair-shared q0)
                                    moff = (j * KB - c * SB - q0) // KB
                                    nc.vector.tensor_mul(
                                        pt2[:, half, q0:],
                                        pt2[:, half, q0:],
                                        msk_sb[:, moff, 0 : SB - q0],
                                    )
                            self.pend.append((j0, pt2[:, 0, :], q0))
                            self.pend.append((j1, pt2[:, 1, :], q0))
                            while len(self.pend) > depth:
                                self.pv_step(*self.pend.pop(0))

                        def drain(self):
                            for t in self.pend:
                                self.pv_step(*t)
                            self.pend = []

                    # heads 0 and 1 interleave pair-by-pair; head 2 runs
                    # alone with a deeper private pipeline. background
                    # projection chunks are spread across the whole block.
                    s0, s1 = HeadStream(0), HeadStream(1)
                    for jp in range(npr):
                        s0.pair(jp, 2)
                        bg_step()
                        s1.pair(jp, 2)
                        bg_step()
                    s0.drain()
                    s1.drain()
                    norm_chain(0, s0.ps_o)
                    s2 = HeadStream(2)
                    for jp in range(npr):
                        s2.pair(jp, 6)
                        bg_step()
                    norm_chain(1, s1.ps_o)
                    bg_flush()
                    s2.drain()
                    norm_chain(2, s2.ps_o)
                    oS_prev = oS_now
                    qS_cur = qS_next if c + 1 < nsb else qS_cur
                for ch in phase3_chunks(nsb - 1, oS_prev):
                    ch()
    nc.compile()
    return nc


def make_in_maps(x, wq, bq, wk, bk, wv, bv, wo, bo, t_len=T):
    x = np.asarray(x, np.float32)
    in_maps = []
    for c in range(8):
        b, g = divmod(c, 4)
        hs = slice(g * CH, (g + 1) * CH)
        wqk_c = np.empty((E, 2 * CH), np.float32)
        bqk_c = np.empty((HL, 2, D), np.float32)
        for hl in range(HL):
            h = g * HL + hl
            wqk_c[:, hl * 128 : hl * 128 + D] = wq[:, h * D : (h + 1) * D]
            wqk_c[:, hl * 128 + D : (hl + 1) * 128] = wk[:, h * D : (h + 1) * D]
            bqk_c[hl, 0] = bq[h * D : (h + 1) * D]
            bqk_c[hl, 1] = bk[h * D : (h + 1) * D]
        wv_c = np.zeros((E, 256), np.float32)
        wv_c[:, :CH] = np.asarray(wv, np.float32)[:, hs]
        cst = np.concatenate(
            [np.eye(128, dtype=np.float32), np.ones((128, 64), np.float32)], axis=1
        )
        col = np.arange(SB)[None, :]
        k = np.arange(128)[:, None]
        msk_np = np.stack(
            [(col >= k), (col >= k + 128)], axis=1
        ).astype(np.float16)
        in_maps.append(
            {
                "xb": np.ascontiguousarray(x[b, :t_len]),
                "wqk": wqk_c,
                "wvp": wv_c,
                "wo": np.ascontiguousarray(np.asarray(wo, np.float32)[hs]),
                "bqk": bqk_c,
                "cst": cst,
                "msk": msk_np,
            }
        )
    return in_maps


def host_combine(y_per_core, bo, bv, wo):
    """Sum the 4 per-core partial y's per batch; add bo and the v-bias
    contribution bv @ wo (constant after softmax normalization)."""
    bias_row = (
        np.asarray(bv, np.float32) @ np.asarray(wo, np.float32)
        + np.asarray(bo, np.float32)
    )
    out = np.empty((B, T, E), np.float32)
    for b in range(B):
        acc = y_per_core[b * 4].astype(np.float32).copy()
        for g in range(1, 4):
            acc += y_per_core[b * 4 + g]
        out[b] = acc + bias_row
    return out


_NC_CACHE = {}


def get_nc(t_len=T):
    if t_len not in _NC_CACHE:
        _NC_CACHE[t_len] = build_nc(t_len)
    return _NC_CACHE[t_len]


def _build_sharded_nodonate(nc, n_cores=8):
    """Mirror bass2jax.run_bass_via_pjrt's multi-core path, minus donation,
    returning (jitted_fn, in_names, out_names, out_avals). Without donation a
    call can be repeated on device-resident arrays for timing. Safe here: the
    kernel writes every element of y."""
    import jax
    from jax.sharding import Mesh, PartitionSpec
    from jax.experimental.shard_map import shard_map

    from concourse import bass2jax
    from concourse.bass2jax import _bass_exec_p

    bass2jax.install_neuronx_cc_hook()
    part_name = nc.partition_id_tensor.name if nc.partition_id_tensor else None

    in_names, out_names, out_avals = [], [], []
    for alloc in nc.m.functions[0].allocations:
        if not isinstance(alloc, mybir.MemoryLocationSet):
            continue
        name = alloc.memorylocations[0].name
        if alloc.kind == "ExternalInput":
            if name != part_name:
                in_names.append(name)
        elif alloc.kind == "ExternalOutput":
            shape = tuple(alloc.tensor_shape)
            dtype = mybir.dt.np(alloc.dtype)
            out_names.append(name)
            out_avals.append(jax.core.ShapedArray(shape, dtype))
    n_params = len(in_names)
    all_names = in_names + out_names
    if part_name is not None:
        all_names = all_names + [part_name]

    def _body(*args):
        operands = list(args)
        if part_name is not None:
            operands.append(bass2jax.partition_id_tensor())
        outs = _bass_exec_p.bind(
            *operands,
            out_avals=tuple(out_avals),
            in_names=tuple(all_names),
            out_names=tuple(out_names),
            lowering_input_output_aliases=(),
            sim_require_finite=True,
            sim_require_nnan=True,
            nc=nc,
        )
        return tuple(outs)

    devices = jax.devices()[:n_cores]
    mesh = Mesh(np.asarray(devices), ("core",))
    n_out = len(out_names)
    sharded = jax.jit(
        shard_map(
            _body,
            mesh=mesh,
            in_specs=(PartitionSpec("core"),) * (n_params + n_out),
            out_specs=(PartitionSpec("core"),) * n_out,
            check_rep=False,
        ),
        keep_unused=True,
    )
    return sharded, in_names, out_names, out_avals


def run_timed(nc, in_maps, iters=20):
    """Execute on HW repeatedly with device-resident args; returns
    (per-core results, sorted per-call walls in seconds)."""
    import time

    import jax

    n_cores = len(in_maps)
    sharded, in_names, out_names, out_avals = _build_sharded_nodonate(nc, n_cores)
    concat_in = [
        np.concatenate([np.asarray(m[name]) for m in in_maps], axis=0)
        for name in in_names
    ]
    concat_zero = [
        np.zeros((n_cores * a.shape[0], *a.shape[1:]), a.dtype) for a in out_avals
    ]
    args = [jax.device_put(a) for a in concat_in + concat_zero]
    out = sharded(*args)  # compile + first run
    jax.block_until_ready(out)
    walls = []
    for _ in range(iters):
        t0 = time.perf_counter()
        out2 = sharded(*args)
        jax.block_until_ready(out2)
        walls.append(time.perf_counter() - t0)
    results = [
        {
            name: np.asarray(out[i]).reshape(n_cores, *out_avals[i].shape)[c]
            for i, name in enumerate(out_names)
        }
        for c in range(n_cores)
    ]
    return results, sorted(walls)


def baseline_rtt(iters=20):
    """Axon dispatch floor: same path with a trivial 8-core kernel."""
    nc = bacc.Bacc("TRN2", target_bir_lowering=False, debug=False, num_devices=8)
    a = nc.dram_tensor("a", [128, 128], F32, kind="ExternalInput")
    b = nc.dram_tensor("b", [128, 128], F32, kind="ExternalOutput")
    a, b = a.ap(), b.ap()
    with tile.TileContext(nc) as tc:
        with tc.tile_pool(name="p", bufs=1) as p:
            t = p.tile([128, 128], F32)
            nc.sync.dma_start(out=t, in_=a)
            nc.scalar.mul(out=t, in_=t, mul=2.0)
            nc.sync.dma_start(out=b, in_=t)
    nc.compile()
    in_maps = [{"a": np.zeros((128, 128), np.float32)} for _ in range(8)]
    _, walls = run_timed(nc, in_maps, iters=iters)
    return walls


def kernel(x, wq, bq, wk, bk, wv, bv, wo, bo, _trace=False, _trace_kwargs=None):
    nc = get_nc()
    in_maps = make_in_maps(x, wq, bq, wk, bk, wv, bv, wo, bo)
    res = run_bass_kernel_spmd(
        nc, in_maps, list(range(8)), trace=_trace, **(_trace_kwargs or {})
    )
    out = host_combine([res.results[c]["y"] for c in range(8)], bo, bv, wo)
    if _trace:
        return out, res
    return out
